# revision 2
# baseline (speedup 1.0000x reference)
"""Multi-head attention (B=4, S=2048, E=1024, H=16, D=64) on 8 Trainium2 cores.

Sharding: 8 cores = 4 batches x 2 head-halves (data parallel on B, tensor
parallel on heads: each core handles 8 heads = 512 of the 1024 QKV columns /
out-proj rows for one batch). Each core returns a partial [S, E] output
(its half of the out-projection contraction); the host sums core pairs.

v2 layout (vs the original): ascending q-tile order with per-chunk K/V/Q
projections interleaved into the attention stream (the projection of chunk
t+1 is emitted inside attention on tile t, so the tensor engine never sits
behind an upfront full-K projection), bf16 matmul operands (host-cast; FWL
weight loads + 2x DVE + half DMA), and triangular narrowing of the
score/exp/PV work on diagonal blocks (masked columns are simply never
computed; PSUM has_written semantics make the narrower PV accumulation
exact).

Device algorithm per core (fp32 PSUM accumulation everywhere):
  - QT = Q^T [512, S] in head-pair layout [128, 4, S] (partition =
    d-within-pair); same for KT. V in [128, 16, 8*65] with a ones-column
    per head (softmax denominators ride the PV matmul as row 64).
  - Scores transposed: ST[k, q] = KT-slice^T . QT-slice per (head,
    k-block, q-tile 512); exp on ScalarE with scale=1/8 (|s|/8 stays well
    inside fp32 exp range, so no max-subtraction); two k-blocks per
    activation.
  - Causal masking: diagonal 128x128 block multiplied by a 0/1 triangle;
    columns left of the block are never computed (narrowed N).
  - PV: OT_aug[65, q] += V_aug^T . PT accumulated over k in PSUM; row 64 =
    denominator; reciprocal + K=1 broadcast matmul + multiply normalize.
  - Out-projection: out[q, e] += OT_pair_j^T . wo rows.
"""

import os
import sys

sys.path.insert(0, "/opt/trn_rl_repo")

import numpy as np

B, S, E, H = 4, 2048, 1024, 16
D = E // H  # 64
P = 128
KO = E // P          # 8 contraction chunks for projections
NJ = 4               # head pairs per core
SQT = 512            # q tile
NQT = S // SQT       # 4
NKB = S // P         # 16 k blocks
EH = E // 2          # 512 columns per core

_CACHE = {}
LAST_RESULT = None


def _build(causal: bool, dtype: str = "bf16", repeat: int = 1,
           phases=("proj", "attn")):
    import concourse.bass as bass  # noqa: F401
    import concourse.mybir as mybir
    import concourse.tile as tile
    from concourse import bacc
    from contextlib import ExitStack

    f32 = mybir.dt.float32
    if dtype == "bf16":
        rdt = mybir.dt.bfloat16
        in_dt = mybir.dt.bfloat16
    elif dtype == "f32r":
        rdt = mybir.dt.float32r
        in_dt = f32
    else:
        rdt = in_dt = f32
    AF = mybir.ActivationFunctionType

    nc = bacc.Bacc("TRN2", target_bir_lowering=False, debug=False, num_devices=8)

    xt_q = nc.dram_tensor("xt_q", [E, S], in_dt, kind="ExternalInput")
    xt_k = nc.dram_tensor("xt_k", [E, S], in_dt, kind="ExternalInput")
    xt_v = nc.dram_tensor("xt_v", [E, S], in_dt, kind="ExternalInput")
    wq_d = nc.dram_tensor("wq_h", [E, EH], in_dt, kind="ExternalInput")
    wk_d = nc.dram_tensor("wk_h", [E, EH], in_dt, kind="ExternalInput")
    wv_d = nc.dram_tensor("wv_h", [E, EH], in_dt, kind="ExternalInput")
    wo_d = nc.dram_tensor("wo_h", [EH, E], in_dt, kind="ExternalInput")
    tri_d = nc.dram_tensor("tri", [P, P], in_dt, kind="ExternalInput")
    sel_d = nc.dram_tensor("sel2", [2, P], in_dt, kind="ExternalInput")
    out_d = nc.dram_tensor("out", [S, E], f32, kind="ExternalOutput")

    def rcast(ap):
        return ap.bitcast(rdt) if dtype == "f32r" else ap

    with nc.allow_low_precision(reason="low precision matmul inputs"), \
            tile.TileContext(nc) as tc, ExitStack() as top:
        bf16 = dtype == "bf16"
        consts = top.enter_context(tc.tile_pool(name="consts", bufs=1))
        big = top.enter_context(tc.tile_pool(name="big", bufs=1))
        xtp = top.enter_context(tc.tile_pool(name="xtp", bufs=3 if bf16 else 2))
        wp = top.enter_context(tc.tile_pool(name="wp", bufs=2 if bf16 else 1))
        # causal: double-buffered q-tile prefetch; non-causal: all q chunks
        # stay live because every projection is emitted before any attention
        qtp = top.enter_context(tc.tile_pool(name="qtp", bufs=2 if causal else 5))
        ptp = top.enter_context(tc.tile_pool(name="ptp", bufs=3))
        repp = top.enter_context(tc.tile_pool(name="repp", bufs=2 if bf16 else 1))
        # ot tiles stay live until the deferred out-projection drains during
        # the NEXT tile's attention: up to 2 q-tiles' worth in flight.
        # f32r mode is SBUF-tight: drop to 6 (out-projections drain earlier)
        otp = top.enter_context(tc.tile_pool(name="otp", bufs=10 if bf16 else 6))
        osbp = top.enter_context(tc.tile_pool(name="osbp", bufs=3))
        # PSUM budget (8 banks): st0/st1 bank-pairs = 4, pv0/pv1 = 2,
        # shared "mm" tag (projection groups, rep broadcast, out-proj) = 2.
        st_ps = top.enter_context(tc.tile_pool(name="st_ps", bufs=1, space="PSUM"))
        pv_ps = top.enter_context(tc.tile_pool(name="pv_ps", bufs=1, space="PSUM"))
        mm_ps = top.enter_context(tc.tile_pool(name="mm_ps", bufs=2, space="PSUM"))

        tri_sb = consts.tile([P, P], rdt, tag="tri")
        nc.sync.dma_start(tri_sb[:], rcast(tri_d.ap()))
        wo_sb = consts.tile([P, NJ, E], rdt, tag="wo")

        kt_sb = big.tile([P, NJ, S], rdt, tag="KT")
        # V head groups padded to 128 columns (64 V + 1 ones + 63 zeros) so
        # the PV weight loads are exactly 128 wide and take the FWL fast path
        v_sb = big.tile([P, NKB, 8 * P], rdt, tag="V")
        nc.gpsimd.memset(v_sb[:], 0.0)
        v_view = v_sb[:].rearrange("p b (h w) -> p b h w", h=8)
        nc.gpsimd.memset(v_view[:, :, :, D : D + 1], 1.0)
        if "proj" not in phases:  # timing-probe mode: fill inputs of attn
            nc.gpsimd.memset(kt_sb[:], 0.01)
            nc.gpsimd.memset(v_sb[:], 0.01)

        def load_w(w_dram, tag, interleave_with=None):
            """DMA a weight [E, EH] into a rotating tile. If
            interleave_with=(xt_dram, sc), alternate weight and xt chunk DMAs
            so the first matmul group can start after ~2 chunks."""
            w_t = wp.tile([P, KO, EH], rdt, tag=tag, name=tag)
            xt_t = None
            if interleave_with is not None:
                xt_dram, sc = interleave_with
                xt_t = xtp.tile([P, KO, SQT], rdt, tag="xt", name="xt_t")
            for ko in range(KO):
                nc.sync.dma_start(
                    w_t[:, ko, :],
                    rcast(w_dram.ap()[ko * P : (ko + 1) * P, :]),
                )
                if xt_t is not None:
                    nc.sync.dma_start(
                        xt_t[:, ko, :],
                        rcast(
                            xt_dram.ap()[
                                ko * P : (ko + 1) * P, sc * SQT : (sc + 1) * SQT
                            ]
                        ),
                    )
            return w_t, xt_t

        def load_xt(xt_dram, sc):
            xt_t = xtp.tile([P, KO, SQT], rdt, tag="xt", name="xt_t")
            for ko in range(KO):
                nc.sync.dma_start(
                    xt_t[:, ko, :],
                    rcast(
                        xt_dram.ap()[
                            ko * P : (ko + 1) * P, sc * SQT : (sc + 1) * SQT
                        ]
                    ),
                )
            return xt_t

        def proj_dt_j(w_t, xt_t, dst, dst_col, j):
            """One [d-pair, 512] chunk of QT or KT (head pair j)."""
            pst = mm_ps.tile([P, SQT], f32, tag="mm", name="pst")
            for ko in range(KO):
                nc.tensor.matmul(
                    pst[:],
                    w_t[:, ko, j * P : (j + 1) * P],
                    xt_t[:, ko, :],
                    start=(ko == 0),
                    stop=(ko == KO - 1),
                )
            nc.vector.tensor_copy(dst[:, j, dst_col : dst_col + SQT], pst[:])

        def proj_v_j(w_t, xt_t, sc, sb):
            pst = mm_ps.tile([P, EH], f32, tag="mm", name="pst")
            for ko in range(KO):
                nc.tensor.matmul(
                    pst[:],
                    xt_t[:, ko, sb * P : (sb + 1) * P],
                    w_t[:, ko, :],
                    start=(ko == 0),
                    stop=(ko == KO - 1),
                )
            sblk = 4 * sc + sb
            nc.vector.tensor_copy(
                v_sb[:, sblk, :].rearrange("p (h w) -> p h w", h=8)[:, :, 0:D],
                pst[:].rearrange("p (h w) -> p h w", h=8),
            )

        projq = []  # next-chunk projection thunks (deadline: next tile)
        oprojq = []  # deferred out-projection groups (no deadline)

        def drain(n):
            for _ in range(n):
                if projq:
                    projq.pop(0)()
                elif oprojq:
                    oprojq.pop(0)()

        def attn_qt(qt, qt_t):
            """Attention for q-tile qt; drains the deferred-work queues
            between steps so the scheduler can fill tensor-engine gaps.
            Everything this tile reads (projections of chunks <= qt) was
            already emitted: projq held only chunk qt's work at entry and
            is force-drained first."""
            nkb = 4 * (qt + 1) if causal else NKB
            ot_tiles = []
            for j in range(NJ):
                pv = [
                    pv_ps.tile([P, SQT], f32, tag=f"pv{h2}", name=f"pv{h2}")
                    for h2 in (0, 1)
                ]
                for kbp in range(nkb // 2):
                    kbs = (2 * kbp, 2 * kbp + 1)
                    for h2 in (0, 1):
                        h = 2 * j + h2
                        # column offset (units of P) of the first valid q
                        # column for each kb in this pair (causal narrowing)
                        offs = [
                            max(0, kb - 4 * qt) if causal else 0 for kb in kbs
                        ]
                        st = st_ps.tile(
                            [P, 2 * SQT], f32, tag=f"st{h2}", name=f"st{h2}"
                        )
                        for i, kb in enumerate(kbs):
                            o = offs[i] * P
                            nc.tensor.matmul(
                                st[:, i * SQT + o : (i + 1) * SQT],
                                kt_sb[
                                    h2 * D : (h2 + 1) * D,
                                    j,
                                    kb * P : (kb + 1) * P,
                                ],
                                qt_t[h2 * D : (h2 + 1) * D, j, o:SQT],
                                start=True,
                                stop=True,
                                tile_position=(h2 * D, 0),
                            )
                        pt = ptp.tile(
                            [P, 2 * SQT], rdt, tag=f"pt{h2}", name=f"pt{h2}"
                        )
                        if offs[0] == offs[1]:
                            # same offset (0,0 in the common case): one
                            # contiguous exp over both k-blocks
                            o = offs[0] * P
                            nc.scalar.activation(
                                pt[:, o : 2 * SQT], st[:, o : 2 * SQT],
                                AF.Exp, scale=0.125,
                            )
                        else:
                            for i in range(2):
                                o = offs[i] * P
                                nc.scalar.activation(
                                    pt[:, i * SQT + o : (i + 1) * SQT],
                                    st[:, i * SQT + o : (i + 1) * SQT],
                                    AF.Exp, scale=0.125,
                                )
                        for i, kb in enumerate(kbs):
                            o = offs[i] * P
                            ptk = pt[:, i * SQT : (i + 1) * SQT]
                            if causal and kb >= 4 * qt:
                                # 0/1 triangle on the diagonal 128x128 block
                                nc.vector.tensor_mul(
                                    ptk[:, o : o + P],
                                    ptk[:, o : o + P],
                                    tri_sb[:],
                                )
                            nc.tensor.matmul(
                                pv[h2][:, o:SQT],
                                v_sb[:, kb, h * P : (h + 1) * P],
                                ptk[:, o:SQT],
                                start=(kb == 0),
                                stop=(kb == nkb - 1),
                            )
                    drain(1)
                # normalize: reciprocal of the denominator row (read straight
                # from PSUM), broadcast on the otherwise-idle GpSimd engine,
                # then scale the PV rows (PSUM read) into the ot tile
                ot = otp.tile([P, SQT], rdt, tag="ot", name="ot")
                ot_tiles.append(ot)
                for h2 in (0, 1):
                    den = repp.tile([1, SQT], f32, tag="den", name=f"den{h2}")
                    nc.vector.reciprocal(den[:], pv[h2][D : D + 1, :])
                    rb = repp.tile([P, SQT], f32, tag="rep", name=f"rb{h2}")
                    nc.gpsimd.partition_broadcast(rb[:], den[:])
                    nc.vector.tensor_mul(
                        ot[h2 * D : (h2 + 1) * D, :],
                        pv[h2][0:D, :],
                        rb[h2 * D : (h2 + 1) * D, :],
                    )
                drain(1)
            # out-projection for this q-tile is deferred: its groups fill
            # tensor-engine gaps during later tiles' attention
            for qb in range(4):
                for ec in range(2):
                    def oproj(qt=qt, qb=qb, ec=ec, ot_tiles=ot_tiles):
                        ops = mm_ps.tile([P, SQT], f32, tag="mm", name="ops")
                        for j in range(NJ):
                            nc.tensor.matmul(
                                ops[:],
                                ot_tiles[j][:, qb * P : (qb + 1) * P],
                                wo_sb[:, j, ec * SQT : (ec + 1) * SQT],
                                start=(j == 0),
                                stop=(j == NJ - 1),
                            )
                        osb = osbp.tile([P, SQT], f32, tag="osb", name="osb")
                        nc.vector.tensor_copy(osb[:], ops[:])
                        nc.sync.dma_start(
                            out_d.ap()[
                                qt * SQT + qb * P : qt * SQT + (qb + 1) * P,
                                ec * SQT : (ec + 1) * SQT,
                            ],
                            osb[:],
                        )
                    oprojq.append(oproj)

        def make_feed(wk_t, wv_t, wq_t, sc):
            """Projection thunks for chunk sc. Q first: it gates the very
            first scores of the next q-tile; K/V chunks are only read later."""
            state = {}

            def ld(which, dram):
                def f():
                    state[which] = load_xt(dram, sc)
                return f

            thunks = [ld("xq", xt_q)]
            qt_t = qtp.tile([P, NJ, SQT], rdt, tag="qt", name="qt_t")
            for j in range(NJ):
                def fq(j=j):
                    proj_dt_j(wq_t, state["xq"], qt_t, 0, j)
                thunks.append(fq)
            thunks.append(ld("xk", xt_k))
            for j in range(NJ):
                def fk(j=j):
                    proj_dt_j(wk_t, state["xk"], kt_sb, sc * SQT, j)
                thunks.append(fk)
            thunks.append(ld("xv", xt_v))
            for sb in range(4):
                def fv(sb=sb):
                    proj_v_j(wv_t, state["xv"], sc, sb)
                thunks.append(fv)
            return thunks, qt_t

        for _rep in range(repeat):
            if "proj" in phases:
                wq_t, xtq0 = load_w(wq_d, "wq", interleave_with=(xt_q, 0))
                qt_cur = qtp.tile([P, NJ, SQT], rdt, tag="qt", name="qt_t")
                for j in range(NJ):
                    proj_dt_j(wq_t, xtq0, qt_cur, 0, j)
                wk_t, xtk0 = load_w(wk_d, "wk", interleave_with=(xt_k, 0))
                wv_t, xtv0 = load_w(wv_d, "wv", interleave_with=(xt_v, 0))
                for j in range(NJ):
                    proj_dt_j(wk_t, xtk0, kt_sb, 0, j)
                for sb in range(4):
                    proj_v_j(wv_t, xtv0, 0, sb)
            if _rep == 0:
                # wo is only read by the (deferred) out-projection of tile 0,
                # drained during attention on tile 1 — keep its DMA off the
                # critical startup path
                nc.sync.dma_start(
                    wo_sb[:], rcast(wo_d.ap().rearrange("(j p) e -> p j e", p=P))
                )
            qnext = {}
            if "proj" in phases and not causal:
                # non-causal attention on tile 0 already reads ALL k chunks,
                # so every projection must be emitted before any attention
                # (Tile dependencies follow program order)
                for sc in range(1, NQT):
                    feed, qnext[sc] = make_feed(wk_t, wv_t, wq_t, sc)
                    for f in feed:
                        f()
            for t in range(NQT):
                # deadline: everything attention on tile t reads (chunk t's
                # projections, fed during tile t-1) must be emitted by now
                while projq:
                    projq.pop(0)()
                if "proj" in phases and causal and t + 1 < NQT:
                    feed, qt_next = make_feed(wk_t, wv_t, wq_t, t + 1)
                    projq.extend(feed)
                else:
                    qt_next = qnext.get(t + 1)
                if "attn" in phases:
                    if "proj" not in phases:
                        qt_cur = qtp.tile([P, NJ, SQT], rdt, tag="qt", name="qt_t")
                        nc.gpsimd.memset(qt_cur[:], 0.01)
                    attn_qt(t, qt_cur)
                else:
                    drain(len(projq) + len(oprojq))
                qt_cur = qt_next
            drain(len(projq) + len(oprojq))

    nc.compile()
    return nc


def _dtype_default():
    return os.environ.get("KDTYPE", "bf16")


def _get_nc(causal: bool):
    key = (causal, _dtype_default())
    if key not in _CACHE:
        _CACHE[key] = _build(causal, dtype=key[1])
    return _CACHE[key]


def _numpy_ref(query, key, value, mask, wq, bq, wk, bk, wv, bv, wo, bo):
    """Exact fallback for inputs the device kernel doesn't specialize."""
    q = (query @ wq + bq).reshape(B, S, H, D).transpose(0, 2, 1, 3)
    k = (key @ wk + bk).reshape(B, S, H, D).transpose(0, 2, 1, 3)
    v = (value @ wv + bv).reshape(B, S, H, D).transpose(0, 2, 1, 3)
    out = np.empty((B, H, S, D), np.float32)
    for b in range(B):
        for h in range(H):
            s = q[b, h] @ k[b, h].T
            s = np.where(mask[b, 0], s, -np.inf) / np.sqrt(np.float32(D))
            s = s - s.max(axis=-1, keepdims=True)
            e = np.exp(s)
            out[b, h] = (e / e.sum(axis=-1, keepdims=True)) @ v[b, h]
    out = out.transpose(0, 2, 1, 3).reshape(B, S, E)
    return (out @ wo + bo).astype(np.float32)


def _np_dt(dtype):
    if dtype == "bf16":
        import ml_dtypes

        return ml_dtypes.bfloat16
    return np.float32


def _make_in_maps(query, key, value, wq, wk, wv, wo, dtype=None):
    dtype = dtype or _dtype_default()
    ndt = _np_dt(dtype)
    tri = np.ascontiguousarray(np.triu(np.ones((P, P), ndt)))
    sel2 = np.zeros((2, P), ndt)
    sel2[0, 0:D] = 1.0
    sel2[1, D:P] = 1.0
    in_maps = []
    for b in range(B):
        xq = np.ascontiguousarray(query[b].T.astype(ndt))
        xk = np.ascontiguousarray(key[b].T.astype(ndt))
        xv = np.ascontiguousarray(value[b].T.astype(ndt))
        for half in (0, 1):
            cs = slice(half * EH, (half + 1) * EH)
            in_maps.append(
                {
                    "xt_q": xq,
                    "xt_k": xk,
                    "xt_v": xv,
                    "wq_h": np.ascontiguousarray(wq[:, cs]).astype(ndt),
                    "wk_h": np.ascontiguousarray(wk[:, cs]).astype(ndt),
                    "wv_h": np.ascontiguousarray(wv[:, cs]).astype(ndt),
                    "wo_h": np.ascontiguousarray(wo[cs, :]).astype(ndt),
                    "tri": tri,
                    "sel2": sel2,
                }
            )
    return in_maps


def kernel(query, key, value, mask, wq, bq, wk, bk, wv, bv, wo, bo):
    global LAST_RESULT
    query = np.asarray(query, np.float32)
    key = np.asarray(key, np.float32)
    value = np.asarray(value, np.float32)
    mask = np.asarray(mask)

    biases_zero = not (np.any(bq) or np.any(bk) or np.any(bv) or np.any(bo))
    m0 = mask[0, 0]
    tril = np.tril(np.ones((S, S), bool))
    if np.array_equal(m0, tril) and all(
        np.array_equal(mask[b, 0], m0) for b in range(1, B)
    ):
        causal = True
    elif mask.all():
        causal = False
    else:
        causal = None
    if query.shape != (B, S, E) or not biases_zero or causal is None:
        return _numpy_ref(
            query, key, value, mask, wq, bq, wk, bk, wv, bv, wo, bo
        )

    from concourse import bass_utils

    in_maps = _make_in_maps(query, key, value, wq, wk, wv, wo)
    nc = _get_nc(causal)
    res = bass_utils.run_bass_kernel_spmd(
        nc, in_maps, core_ids=list(range(8))
    )
    LAST_RESULT = res
    out = np.empty((B, S, E), np.float32)
    for b in range(B):
        out[b] = res.results[2 * b]["out"] + res.results[2 * b + 1]["out"]
    return out


def benchmark(query, key, value, mask, wq, bq, wk, bk, wv, bv, wo, bo, iters=10):
    """Time repeated on-device executions with device-resident inputs.

    Returns (per_iter_seconds, outputs_like_kernel). Dispatch overhead through
    the axon tunnel is large (~10ms+), so this is an upper bound only.
    """
    import time
    import jax
    from jax.sharding import Mesh, PartitionSpec, NamedSharding
    from jax.experimental.shard_map import shard_map
    import concourse.mybir as mybir
    from concourse.bass2jax import (
        _bass_exec_p,
        install_neuronx_cc_hook,
        partition_id_tensor,
    )

    install_neuronx_cc_hook()
    query = np.asarray(query, np.float32)
    key = np.asarray(key, np.float32)
    value = np.asarray(value, np.float32)
    in_maps = _make_in_maps(query, key, value, wq, wk, wv, wo)
    nc = _get_nc(True)
    n_cores = 8

    partition_name = nc.partition_id_tensor.name if nc.partition_id_tensor else None
    in_names, out_names, out_avals, zero_outs = [], [], [], []
    for alloc in nc.m.functions[0].allocations:
        if not isinstance(alloc, mybir.MemoryLocationSet):
            continue
        name = alloc.memorylocations[0].name
        if alloc.kind == "ExternalInput":
            if name != partition_name:
                in_names.append(name)
        elif alloc.kind == "ExternalOutput":
            shape = tuple(alloc.tensor_shape)
            dtype = mybir.dt.np(alloc.dtype)
            out_names.append(name)
            out_avals.append(jax.core.ShapedArray(shape, dtype))
            zero_outs.append(np.zeros(shape, dtype))
    n_params = len(in_names)
    n_outs = len(out_avals)
    all_in_names = list(in_names) + out_names
    if partition_name is not None:
        all_in_names.append(partition_name)

    def _body(*args):
        operands = list(args)
        if partition_name is not None:
            operands.append(partition_id_tensor())
        return tuple(
            _bass_exec_p.bind(
                *operands,
                out_avals=tuple(out_avals),
                in_names=tuple(all_in_names),
                out_names=tuple(out_names),
                lowering_input_output_aliases=(),
                sim_require_finite=True,
                sim_require_nnan=True,
                nc=nc,
            )
        )

    devices = jax.devices()[:n_cores]
    mesh = Mesh(np.asarray(devices), ("core",))
    sharded = jax.jit(
        shard_map(
            _body,
            mesh=mesh,
            in_specs=(PartitionSpec("core"),) * (n_params + n_outs),
            out_specs=(PartitionSpec("core"),) * n_outs,
            check_rep=False,
        ),
        donate_argnums=tuple(range(n_params, n_params + n_outs)),
        keep_unused=True,
    )
    sh = NamedSharding(mesh, PartitionSpec("core"))
    concat_in = [
        jax.device_put(
            np.concatenate([np.asarray(in_maps[c][nm]) for c in range(n_cores)], 0), sh
        )
        for nm in in_names
    ]

    def fresh_zeros():
        return [
            jax.device_put(np.zeros((n_cores * z.shape[0], *z.shape[1:]), z.dtype), sh)
            for z in zero_outs
        ]

    outs = sharded(*concat_in, *fresh_zeros())
    jax.block_until_ready(outs)
    zsets = [fresh_zeros() for _ in range(iters)]
    for zs in zsets:
        jax.block_until_ready(zs)
    t0 = time.time()
    res = [sharded(*concat_in, *zs) for zs in zsets]
    jax.block_until_ready(res)
    dt = (time.time() - t0) / iters
    out_np = np.asarray(res[-1][out_names.index("out")]).reshape(n_cores, S, E)
    out = np.empty((B, S, E), np.float32)
    for b in range(B):
        out[b] = out_np[2 * b] + out_np[2 * b + 1]
    return dt, out


# revision 3
# speedup vs baseline: 1.1563x; 1.1563x over previous
"""Multi-head attention (B=4, S=2048, E=1024, H=16, D=64) on 8 Trainium2 cores.

Sharding: 8 cores = 4 batches x 2 head-halves (data parallel on B, tensor
parallel on heads: each core handles 8 heads = 512 of the 1024 QKV columns /
out-proj rows for one batch). Each core returns a partial [S, E] output
(its half of the out-projection contraction); the host sums core pairs.

v2 layout (vs the original): ascending q-tile order with per-chunk K/V/Q
projections interleaved into the attention stream (the projection of chunk
t+1 is emitted inside attention on tile t, so the tensor engine never sits
behind an upfront full-K projection), bf16 matmul operands (host-cast; FWL
weight loads + 2x DVE + half DMA), and triangular narrowing of the
score/exp/PV work on diagonal blocks (masked columns are simply never
computed; PSUM has_written semantics make the narrower PV accumulation
exact).

Device algorithm per core (fp32 PSUM accumulation everywhere):
  - QT = Q^T [512, S] in head-pair layout [128, 4, S] (partition =
    d-within-pair); same for KT. V in [128, 16, 8*128] with head groups
    padded to 128 columns (64 V + ones column + zeros) so PV weight loads
    take the FWL fast path; softmax denominators ride the PV matmul as
    row 64 of each group.
  - Scores transposed: ST[k, q] = KT-slice^T . QT-slice per (head,
    k-block, q-tile 512); exp on ScalarE with scale=1/8 (|s|/8 stays well
    inside fp32 exp range, so no max-subtraction); two k-blocks per
    activation.
  - Causal masking: diagonal 128x128 block multiplied by a 0/1 triangle;
    columns left of the block are never computed (narrowed N).
  - PV: OT_aug[65, q] += V_aug^T . PT accumulated over k in PSUM; row 64 =
    denominator; reciprocal + K=1 broadcast matmul + multiply normalize.
  - Out-projection: out[q, e] += OT_pair_j^T . wo rows.
"""

import os
import sys

sys.path.insert(0, "/opt/trn_rl_repo")

import numpy as np

B, S, E, H = 4, 2048, 1024, 16
D = E // H  # 64
P = 128
KO = E // P          # 8 contraction chunks for projections
NJ = 4               # head pairs per core
SQT = 512            # q tile
NQT = S // SQT       # 4
NKB = S // P         # 16 k blocks
EH = E // 2          # 512 columns per core

_CACHE = {}
LAST_RESULT = None


def _build(causal: bool, dtype: str = "bf16", repeat: int = 1,
           phases=("proj", "attn")):
    import concourse.bass as bass  # noqa: F401
    import concourse.mybir as mybir
    import concourse.tile as tile
    from concourse import bacc
    from contextlib import ExitStack

    f32 = mybir.dt.float32
    if dtype == "bf16":
        rdt = mybir.dt.bfloat16
        in_dt = mybir.dt.bfloat16
    elif dtype == "f32r":
        rdt = mybir.dt.float32r
        in_dt = f32
    else:
        rdt = in_dt = f32
    AF = mybir.ActivationFunctionType

    nc = bacc.Bacc("TRN2", target_bir_lowering=False, debug=False, num_devices=8)

    xt_q = nc.dram_tensor("xt_q", [E, S], in_dt, kind="ExternalInput")
    xt_k = nc.dram_tensor("xt_k", [E, S], in_dt, kind="ExternalInput")
    xt_v = nc.dram_tensor("xt_v", [E, S], in_dt, kind="ExternalInput")
    wq_d = nc.dram_tensor("wq_h", [E, EH], in_dt, kind="ExternalInput")
    wk_d = nc.dram_tensor("wk_h", [E, EH], in_dt, kind="ExternalInput")
    wv_d = nc.dram_tensor("wv_h", [E, EH], in_dt, kind="ExternalInput")
    wo_d = nc.dram_tensor("wo_h", [EH, E], in_dt, kind="ExternalInput")
    tri_d = nc.dram_tensor("tri", [P, P], in_dt, kind="ExternalInput")
    sel_d = nc.dram_tensor("sel2", [2, P], in_dt, kind="ExternalInput")
    out_d = nc.dram_tensor("out", [S, E], f32, kind="ExternalOutput")

    def rcast(ap):
        return ap.bitcast(rdt) if dtype == "f32r" else ap

    with nc.allow_low_precision(reason="low precision matmul inputs"), \
            tile.TileContext(nc) as tc, ExitStack() as top:
        bf16 = dtype == "bf16"
        consts = top.enter_context(tc.tile_pool(name="consts", bufs=1))
        big = top.enter_context(tc.tile_pool(name="big", bufs=1))
        xtp = top.enter_context(tc.tile_pool(name="xtp", bufs=3 if bf16 else 2))
        wp = top.enter_context(tc.tile_pool(name="wp", bufs=2 if bf16 else 1))
        # causal: double-buffered q-tile prefetch; non-causal: all q chunks
        # stay live because every projection is emitted before any attention
        qtp = top.enter_context(tc.tile_pool(name="qtp", bufs=2 if causal else 5))
        ptp = top.enter_context(tc.tile_pool(name="ptp", bufs=3))
        repp = top.enter_context(tc.tile_pool(name="repp", bufs=2 if bf16 else 1))
        # ot tiles stay live until the deferred out-projection drains during
        # the NEXT tile's attention: up to 2 q-tiles' worth in flight.
        # f32r mode is SBUF-tight: drop to 6 (out-projections drain earlier)
        otp = top.enter_context(tc.tile_pool(name="otp", bufs=10 if bf16 else 6))
        osbp = top.enter_context(tc.tile_pool(name="osbp", bufs=3))
        # PSUM budget (8 banks): st0/st1 bank-pairs = 4, pv0/pv1 = 2,
        # shared "mm" tag (projection groups, rep broadcast, out-proj) = 2.
        st_ps = top.enter_context(tc.tile_pool(name="st_ps", bufs=1, space="PSUM"))
        pv_ps = top.enter_context(tc.tile_pool(name="pv_ps", bufs=1, space="PSUM"))
        mm_ps = top.enter_context(tc.tile_pool(name="mm_ps", bufs=2, space="PSUM"))

        tri_sb = consts.tile([P, P], rdt, tag="tri")
        nc.sync.dma_start(tri_sb[:], rcast(tri_d.ap()))
        wo_sb = consts.tile([P, NJ, E], rdt, tag="wo")

        kt_sb = big.tile([P, NJ, S], rdt, tag="KT")
        # V head groups padded to 128 columns (64 V + 1 ones + 63 zeros) so
        # the PV weight loads are exactly 128 wide and take the FWL fast path
        v_sb = big.tile([P, NKB, 8 * P], rdt, tag="V")
        nc.gpsimd.memset(v_sb[:], 0.0)
        v_view = v_sb[:].rearrange("p b (h w) -> p b h w", h=8)
        nc.gpsimd.memset(v_view[:, :, :, D : D + 1], 1.0)
        if "proj" not in phases:  # timing-probe mode: fill inputs of attn
            nc.gpsimd.memset(kt_sb[:], 0.01)
            nc.gpsimd.memset(v_sb[:], 0.01)

        def load_w(w_dram, tag, interleave_with=None):
            """DMA a weight [E, EH] into a rotating tile. If
            interleave_with=(xt_dram, sc), alternate weight and xt chunk DMAs
            so the first matmul group can start after ~2 chunks."""
            w_t = wp.tile([P, KO, EH], rdt, tag=tag, name=tag)
            xt_t = None
            if interleave_with is not None:
                xt_dram, sc = interleave_with
                xt_t = xtp.tile([P, KO, SQT], rdt, tag="xt", name="xt_t")
            for ko in range(KO):
                nc.sync.dma_start(
                    w_t[:, ko, :],
                    rcast(w_dram.ap()[ko * P : (ko + 1) * P, :]),
                )
                if xt_t is not None:
                    nc.sync.dma_start(
                        xt_t[:, ko, :],
                        rcast(
                            xt_dram.ap()[
                                ko * P : (ko + 1) * P, sc * SQT : (sc + 1) * SQT
                            ]
                        ),
                    )
            return w_t, xt_t

        def load_xt(xt_dram, sc):
            xt_t = xtp.tile([P, KO, SQT], rdt, tag="xt", name="xt_t")
            for ko in range(KO):
                nc.sync.dma_start(
                    xt_t[:, ko, :],
                    rcast(
                        xt_dram.ap()[
                            ko * P : (ko + 1) * P, sc * SQT : (sc + 1) * SQT
                        ]
                    ),
                )
            return xt_t

        def proj_dt_j(w_t, xt_t, dst, dst_col, j):
            """One [d-pair, 512] chunk of QT or KT (head pair j)."""
            pst = mm_ps.tile([P, SQT], f32, tag="mm", name="pst")
            for ko in range(KO):
                nc.tensor.matmul(
                    pst[:],
                    w_t[:, ko, j * P : (j + 1) * P],
                    xt_t[:, ko, :],
                    start=(ko == 0),
                    stop=(ko == KO - 1),
                )
            nc.vector.tensor_copy(dst[:, j, dst_col : dst_col + SQT], pst[:])

        def proj_v_j(w_t, xt_t, sc, sb):
            pst = mm_ps.tile([P, EH], f32, tag="mm", name="pst")
            for ko in range(KO):
                nc.tensor.matmul(
                    pst[:],
                    xt_t[:, ko, sb * P : (sb + 1) * P],
                    w_t[:, ko, :],
                    start=(ko == 0),
                    stop=(ko == KO - 1),
                )
            sblk = 4 * sc + sb
            nc.vector.tensor_copy(
                v_sb[:, sblk, :].rearrange("p (h w) -> p h w", h=8)[:, :, 0:D],
                pst[:].rearrange("p (h w) -> p h w", h=8),
            )

        projq = []  # next-chunk projection thunks (deadline: next tile)
        oprojq = []  # deferred out-projection groups (no deadline)

        def drain(n):
            for _ in range(n):
                if projq:
                    projq.pop(0)()
                elif oprojq:
                    oprojq.pop(0)()

        def attn_qt(qt, qt_t):
            """Attention for q-tile qt; drains the deferred-work queues
            between steps so the scheduler can fill tensor-engine gaps.
            Everything this tile reads (projections of chunks <= qt) was
            already emitted: projq held only chunk qt's work at entry and
            is force-drained first."""
            nkb = 4 * (qt + 1) if causal else NKB
            ot_tiles = []
            for j in range(NJ):
                pv = [
                    pv_ps.tile([P, SQT], f32, tag=f"pv{h2}", name=f"pv{h2}")
                    for h2 in (0, 1)
                ]
                for kbp in range(nkb // 2):
                    kbs = (2 * kbp, 2 * kbp + 1)
                    for h2 in (0, 1):
                        h = 2 * j + h2
                        # column offset (units of P) of the first valid q
                        # column for each kb in this pair (causal narrowing)
                        offs = [
                            max(0, kb - 4 * qt) if causal else 0 for kb in kbs
                        ]
                        st = st_ps.tile(
                            [P, 2 * SQT], f32, tag=f"st{h2}", name=f"st{h2}"
                        )
                        for i, kb in enumerate(kbs):
                            o = offs[i] * P
                            nc.tensor.matmul(
                                st[:, i * SQT + o : (i + 1) * SQT],
                                kt_sb[
                                    h2 * D : (h2 + 1) * D,
                                    j,
                                    kb * P : (kb + 1) * P,
                                ],
                                qt_t[h2 * D : (h2 + 1) * D, j, o:SQT],
                                start=True,
                                stop=True,
                                tile_position=(h2 * D, 0),
                            )
                        pt = ptp.tile(
                            [P, 2 * SQT], rdt, tag=f"pt{h2}", name=f"pt{h2}"
                        )
                        if offs[0] == offs[1]:
                            # same offset (0,0 in the common case): one
                            # contiguous exp over both k-blocks
                            o = offs[0] * P
                            nc.scalar.activation(
                                pt[:, o : 2 * SQT], st[:, o : 2 * SQT],
                                AF.Exp, scale=0.125,
                            )
                        else:
                            for i in range(2):
                                o = offs[i] * P
                                nc.scalar.activation(
                                    pt[:, i * SQT + o : (i + 1) * SQT],
                                    st[:, i * SQT + o : (i + 1) * SQT],
                                    AF.Exp, scale=0.125,
                                )
                        for i, kb in enumerate(kbs):
                            o = offs[i] * P
                            ptk = pt[:, i * SQT : (i + 1) * SQT]
                            if causal and kb >= 4 * qt:
                                # 0/1 triangle on the diagonal 128x128 block
                                nc.vector.tensor_mul(
                                    ptk[:, o : o + P],
                                    ptk[:, o : o + P],
                                    tri_sb[:],
                                )
                            nc.tensor.matmul(
                                pv[h2][:, o:SQT],
                                v_sb[:, kb, h * P : (h + 1) * P],
                                ptk[:, o:SQT],
                                start=(kb == 0),
                                stop=(kb == nkb - 1),
                            )
                    drain(1)
                # normalize: reciprocal of the denominator row (read straight
                # from PSUM), broadcast on the otherwise-idle GpSimd engine,
                # then scale the PV rows (PSUM read) into the ot tile
                ot = otp.tile([P, SQT], rdt, tag="ot", name="ot")
                ot_tiles.append(ot)
                for h2 in (0, 1):
                    den = repp.tile([1, SQT], f32, tag="den", name=f"den{h2}")
                    nc.vector.reciprocal(den[:], pv[h2][D : D + 1, :])
                    rb = repp.tile([P, SQT], f32, tag="rep", name=f"rb{h2}")
                    nc.gpsimd.partition_broadcast(rb[:], den[:])
                    nc.vector.tensor_mul(
                        ot[h2 * D : (h2 + 1) * D, :],
                        pv[h2][0:D, :],
                        rb[h2 * D : (h2 + 1) * D, :],
                    )
                drain(1)
            # out-projection for this q-tile is deferred: its groups fill
            # tensor-engine gaps during later tiles' attention
            for qb in range(4):
                for ec in range(2):
                    def oproj(qt=qt, qb=qb, ec=ec, ot_tiles=ot_tiles):
                        ops = mm_ps.tile([P, SQT], f32, tag="mm", name="ops")
                        for j in range(NJ):
                            nc.tensor.matmul(
                                ops[:],
                                ot_tiles[j][:, qb * P : (qb + 1) * P],
                                wo_sb[:, j, ec * SQT : (ec + 1) * SQT],
                                start=(j == 0),
                                stop=(j == NJ - 1),
                            )
                        osb = osbp.tile([P, SQT], f32, tag="osb", name="osb")
                        nc.vector.tensor_copy(osb[:], ops[:])
                        nc.sync.dma_start(
                            out_d.ap()[
                                qt * SQT + qb * P : qt * SQT + (qb + 1) * P,
                                ec * SQT : (ec + 1) * SQT,
                            ],
                            osb[:],
                        )
                    oprojq.append(oproj)

        def make_feed(wk_t, wv_t, wq_t, sc):
            """Projection thunks for chunk sc. Q first: it gates the very
            first scores of the next q-tile; K/V chunks are only read later."""
            state = {}

            def ld(which, dram):
                def f():
                    state[which] = load_xt(dram, sc)
                return f

            thunks = [ld("xq", xt_q)]
            qt_t = qtp.tile([P, NJ, SQT], rdt, tag="qt", name="qt_t")
            for j in range(NJ):
                def fq(j=j):
                    proj_dt_j(wq_t, state["xq"], qt_t, 0, j)
                thunks.append(fq)
            thunks.append(ld("xk", xt_k))
            for j in range(NJ):
                def fk(j=j):
                    proj_dt_j(wk_t, state["xk"], kt_sb, sc * SQT, j)
                thunks.append(fk)
            thunks.append(ld("xv", xt_v))
            for sb in range(4):
                def fv(sb=sb):
                    proj_v_j(wv_t, state["xv"], sc, sb)
                thunks.append(fv)
            return thunks, qt_t

        for _rep in range(repeat):
            if "proj" in phases:
                wq_t, xtq0 = load_w(wq_d, "wq", interleave_with=(xt_q, 0))
                qt_cur = qtp.tile([P, NJ, SQT], rdt, tag="qt", name="qt_t")
                for j in range(NJ):
                    proj_dt_j(wq_t, xtq0, qt_cur, 0, j)
                wk_t, xtk0 = load_w(wk_d, "wk", interleave_with=(xt_k, 0))
                wv_t, xtv0 = load_w(wv_d, "wv", interleave_with=(xt_v, 0))
                for j in range(NJ):
                    proj_dt_j(wk_t, xtk0, kt_sb, 0, j)
                for sb in range(4):
                    proj_v_j(wv_t, xtv0, 0, sb)
            if _rep == 0:
                # wo is only read by the (deferred) out-projection of tile 0,
                # drained during attention on tile 1 — keep its DMA off the
                # critical startup path
                nc.sync.dma_start(
                    wo_sb[:], rcast(wo_d.ap().rearrange("(j p) e -> p j e", p=P))
                )
            qnext = {}
            if "proj" in phases and not causal:
                # non-causal attention on tile 0 already reads ALL k chunks,
                # so every projection must be emitted before any attention
                # (Tile dependencies follow program order)
                for sc in range(1, NQT):
                    feed, qnext[sc] = make_feed(wk_t, wv_t, wq_t, sc)
                    for f in feed:
                        f()
            for t in range(NQT):
                # deadline: everything attention on tile t reads (chunk t's
                # projections, fed during tile t-1) must be emitted by now
                while projq:
                    projq.pop(0)()
                if "proj" in phases and causal and t + 1 < NQT:
                    feed, qt_next = make_feed(wk_t, wv_t, wq_t, t + 1)
                    projq.extend(feed)
                else:
                    qt_next = qnext.get(t + 1)
                if "attn" in phases:
                    if "proj" not in phases:
                        qt_cur = qtp.tile([P, NJ, SQT], rdt, tag="qt", name="qt_t")
                        nc.gpsimd.memset(qt_cur[:], 0.01)
                    attn_qt(t, qt_cur)
                else:
                    drain(len(projq) + len(oprojq))
                qt_cur = qt_next
            drain(len(projq) + len(oprojq))

    nc.compile()
    return nc


def _dtype_default():
    return os.environ.get("KDTYPE", "bf16")


def _get_nc(causal: bool):
    key = (causal, _dtype_default())
    if key not in _CACHE:
        _CACHE[key] = _build(causal, dtype=key[1])
    return _CACHE[key]


def _numpy_ref(query, key, value, mask, wq, bq, wk, bk, wv, bv, wo, bo):
    """Exact fallback for inputs the device kernel doesn't specialize."""
    q = (query @ wq + bq).reshape(B, S, H, D).transpose(0, 2, 1, 3)
    k = (key @ wk + bk).reshape(B, S, H, D).transpose(0, 2, 1, 3)
    v = (value @ wv + bv).reshape(B, S, H, D).transpose(0, 2, 1, 3)
    out = np.empty((B, H, S, D), np.float32)
    for b in range(B):
        for h in range(H):
            s = q[b, h] @ k[b, h].T
            s = np.where(mask[b, 0], s, -np.inf) / np.sqrt(np.float32(D))
            s = s - s.max(axis=-1, keepdims=True)
            e = np.exp(s)
            out[b, h] = (e / e.sum(axis=-1, keepdims=True)) @ v[b, h]
    out = out.transpose(0, 2, 1, 3).reshape(B, S, E)
    return (out @ wo + bo).astype(np.float32)


def _np_dt(dtype):
    if dtype == "bf16":
        import ml_dtypes

        return ml_dtypes.bfloat16
    return np.float32


def _make_in_maps(query, key, value, wq, wk, wv, wo, dtype=None):
    dtype = dtype or _dtype_default()
    ndt = _np_dt(dtype)
    tri = np.ascontiguousarray(np.triu(np.ones((P, P), ndt)))
    sel2 = np.zeros((2, P), ndt)
    sel2[0, 0:D] = 1.0
    sel2[1, D:P] = 1.0
    in_maps = []
    for b in range(B):
        xq = np.ascontiguousarray(query[b].T.astype(ndt))
        xk = np.ascontiguousarray(key[b].T.astype(ndt))
        xv = np.ascontiguousarray(value[b].T.astype(ndt))
        for half in (0, 1):
            cs = slice(half * EH, (half + 1) * EH)
            in_maps.append(
                {
                    "xt_q": xq,
                    "xt_k": xk,
                    "xt_v": xv,
                    "wq_h": np.ascontiguousarray(wq[:, cs]).astype(ndt),
                    "wk_h": np.ascontiguousarray(wk[:, cs]).astype(ndt),
                    "wv_h": np.ascontiguousarray(wv[:, cs]).astype(ndt),
                    "wo_h": np.ascontiguousarray(wo[cs, :]).astype(ndt),
                    "tri": tri,
                    "sel2": sel2,
                }
            )
    return in_maps


def kernel(query, key, value, mask, wq, bq, wk, bk, wv, bv, wo, bo):
    global LAST_RESULT
    query = np.asarray(query, np.float32)
    key = np.asarray(key, np.float32)
    value = np.asarray(value, np.float32)
    mask = np.asarray(mask)

    biases_zero = not (np.any(bq) or np.any(bk) or np.any(bv) or np.any(bo))
    m0 = mask[0, 0]
    tril = np.tril(np.ones((S, S), bool))
    if np.array_equal(m0, tril) and all(
        np.array_equal(mask[b, 0], m0) for b in range(1, B)
    ):
        causal = True
    elif mask.all():
        causal = False
    else:
        causal = None
    if query.shape != (B, S, E) or not biases_zero or causal is None:
        return _numpy_ref(
            query, key, value, mask, wq, bq, wk, bk, wv, bv, wo, bo
        )

    from concourse import bass_utils

    in_maps = _make_in_maps(query, key, value, wq, wk, wv, wo)
    nc = _get_nc(causal)
    res = bass_utils.run_bass_kernel_spmd(
        nc, in_maps, core_ids=list(range(8))
    )
    LAST_RESULT = res
    out = np.empty((B, S, E), np.float32)
    for b in range(B):
        out[b] = res.results[2 * b]["out"] + res.results[2 * b + 1]["out"]
    return out


def benchmark(query, key, value, mask, wq, bq, wk, bk, wv, bv, wo, bo, iters=10):
    """Time repeated on-device executions with device-resident inputs.

    Returns (per_iter_seconds, outputs_like_kernel). Dispatch overhead through
    the axon tunnel is large (~10ms+), so this is an upper bound only.
    """
    import time
    import jax
    from jax.sharding import Mesh, PartitionSpec, NamedSharding
    from jax.experimental.shard_map import shard_map
    import concourse.mybir as mybir
    from concourse.bass2jax import (
        _bass_exec_p,
        install_neuronx_cc_hook,
        partition_id_tensor,
    )

    install_neuronx_cc_hook()
    query = np.asarray(query, np.float32)
    key = np.asarray(key, np.float32)
    value = np.asarray(value, np.float32)
    in_maps = _make_in_maps(query, key, value, wq, wk, wv, wo)
    nc = _get_nc(True)
    n_cores = 8

    partition_name = nc.partition_id_tensor.name if nc.partition_id_tensor else None
    in_names, out_names, out_avals, zero_outs = [], [], [], []
    for alloc in nc.m.functions[0].allocations:
        if not isinstance(alloc, mybir.MemoryLocationSet):
            continue
        name = alloc.memorylocations[0].name
        if alloc.kind == "ExternalInput":
            if name != partition_name:
                in_names.append(name)
        elif alloc.kind == "ExternalOutput":
            shape = tuple(alloc.tensor_shape)
            dtype = mybir.dt.np(alloc.dtype)
            out_names.append(name)
            out_avals.append(jax.core.ShapedArray(shape, dtype))
            zero_outs.append(np.zeros(shape, dtype))
    n_params = len(in_names)
    n_outs = len(out_avals)
    all_in_names = list(in_names) + out_names
    if partition_name is not None:
        all_in_names.append(partition_name)

    def _body(*args):
        operands = list(args)
        if partition_name is not None:
            operands.append(partition_id_tensor())
        return tuple(
            _bass_exec_p.bind(
                *operands,
                out_avals=tuple(out_avals),
                in_names=tuple(all_in_names),
                out_names=tuple(out_names),
                lowering_input_output_aliases=(),
                sim_require_finite=True,
                sim_require_nnan=True,
                nc=nc,
            )
        )

    devices = jax.devices()[:n_cores]
    mesh = Mesh(np.asarray(devices), ("core",))
    sharded = jax.jit(
        shard_map(
            _body,
            mesh=mesh,
            in_specs=(PartitionSpec("core"),) * (n_params + n_outs),
            out_specs=(PartitionSpec("core"),) * n_outs,
            check_rep=False,
        ),
        donate_argnums=tuple(range(n_params, n_params + n_outs)),
        keep_unused=True,
    )
    sh = NamedSharding(mesh, PartitionSpec("core"))
    concat_in = [
        jax.device_put(
            np.concatenate([np.asarray(in_maps[c][nm]) for c in range(n_cores)], 0), sh
        )
        for nm in in_names
    ]

    def fresh_zeros():
        return [
            jax.device_put(np.zeros((n_cores * z.shape[0], *z.shape[1:]), z.dtype), sh)
            for z in zero_outs
        ]

    outs = sharded(*concat_in, *fresh_zeros())
    jax.block_until_ready(outs)
    zsets = [fresh_zeros() for _ in range(iters)]
    for zs in zsets:
        jax.block_until_ready(zs)
    t0 = time.time()
    res = [sharded(*concat_in, *zs) for zs in zsets]
    jax.block_until_ready(res)
    dt = (time.time() - t0) / iters
    out_np = np.asarray(res[-1][out_names.index("out")]).reshape(n_cores, S, E)
    out = np.empty((B, S, E), np.float32)
    for b in range(B):
        out[b] = out_np[2 * b] + out_np[2 * b + 1]
    return dt, out
